# revision 20
# baseline (speedup 1.0000x reference)
"""AlphaIouLoss (alpha=2) distributed Bass kernel for 8 TRN2 NeuronCores.

loss = mean(1 - clip(diag_iou, eps)^2)

The reference builds the full NxN IoU matrix and takes its diagonal; only the
diagonal (elementwise pred[i] vs target[i]) is ever used, so each core computes
IoU for its N/8 = 1024 box pairs and reduces sum(iou^2) per SBUF partition on
the DVE (fused square+reduce via the DVE accumulator), then DMAs the 128
per-partition partials out. The host sums the 8x128 partials during unshard:
loss = 1 - sum(iou^2) / N.

Only the SP (DMA), DVE (compute) and Pool (barrier hub) engines carry kernel
instructions; the PE and Activation streams are stripped from the BIR (and the
Pool barrier counts patched) so the NEFF ships no PE/ACT programs. Nothing
waits on the output DMA: its completion overlaps the fixed NEFF postamble
(per-engine semaphore-reset storm + final barrier) that runs before NRT
reports execution complete.

Sharding: boxes split along N across the 8 cores. Per core the host
interleaves pred/target so SBUF partition p holds pred boxes 8p..8p+7 in cols
0:32 and the matching target boxes in cols 32:64 -> one contiguous 32KB DMA
per core.
"""

import numpy as np

import concourse.bass as bass
import concourse.mybir as mybir
from concourse.bass_utils import run_bass_kernel_spmd

N = 8192
NCORES = 8
SHARD = N // NCORES      # 1024 box pairs per core
P = 128                  # SBUF partitions
J = SHARD // P           # 8 box pairs per partition
COLS = 2 * 4 * J         # 64 f32 per partition (pred 0:32 | target 32:64)

_EPS = 1e-07
_ALPHA = 2.0
_SCALE = 1.0


def _strip_engines(nc, drop=("PE", "Activation")):
    """Remove all instructions of the given engines from the BIR and patch the
    Pool-hub barrier counts (gather/release 4 -> 4-len(drop)). The kernel must
    not use those engines. Also drops the dead const-tile init memsets."""
    f = nc.m.functions[0]
    nleft = 4 - len(drop)
    keep_blocks = []
    for blk in f.blocks:
        keep = []
        for i in blk.instructions:
            eng = str(getattr(i, "engine", "")).replace("EngineType.", "")
            if eng in drop:
                continue
            if type(i).__name__ == "InstMemset":
                continue
            si = getattr(i, "sync_info", None)
            if si is not None and eng == "Pool" and type(i).__name__ == "InstEventSemaphore":
                for u in si.on_update or []:
                    if u.update_value == 4:
                        u.update_value = nleft
                for w in si.on_wait or []:
                    if w.wait_value == 4:
                        w.wait_value = nleft
            keep.append(i)
        blk.instructions = keep
        if keep:
            keep_blocks.append(blk)
    f.blocks = keep_blocks
    return nc


def build_bass(strip=True):
    sub = mybir.AluOpType.subtract
    add = mybir.AluOpType.add
    mult = mybir.AluOpType.mult
    amax = mybir.AluOpType.max
    amin = mybir.AluOpType.min
    byp = mybir.AluOpType.bypass
    f32 = mybir.dt.float32

    nc = bass.Bass()
    x_ext = nc.declare_dram_parameter("x", [P, COLS], f32, isOutput=False)
    out_ext = nc.declare_dram_parameter("out", [P, 1], f32, isOutput=True)

    with (
        nc.sbuf_tensor("B", [P, COLS], f32) as B,
        nc.sbuf_tensor("WH", [P, 32], f32) as WH,
        nc.sbuf_tensor("AREA", [P, 16], f32) as AREA,
        nc.sbuf_tensor("LT", [P, 16], f32) as LT,
        nc.sbuf_tensor("RB", [P, 16], f32) as RB,
        nc.sbuf_tensor("D", [P, 16], f32) as D,
        nc.sbuf_tensor("W", [P, 16], f32) as W,
        nc.sbuf_tensor("INTER", [P, J], f32) as INTER,
        nc.sbuf_tensor("S", [P, J], f32) as S,
        nc.sbuf_tensor("UNION", [P, J], f32) as UNION,
        nc.sbuf_tensor("R", [P, J], f32) as R,
        nc.sbuf_tensor("IOU", [P, J], f32) as IOU,
        nc.sbuf_tensor("SQ", [P, J], f32) as SQ,
        nc.sbuf_tensor("ACC", [P, 1], f32) as ACC,
        nc.semaphore("dma_sem") as dma_sem,
        nc.semaphore("v_sem") as v_sem,
        nc.Block() as block,
    ):

        @block.sync
        def _(sync):
            sync.dma_start(out=B[:, :], in_=x_ext[:, :]).then_inc(dma_sem, 16)
            sync.wait_ge(v_sem, 1)
            # No completion wait: the write lands during the fixed NEFF
            # postamble that runs before NRT reports execution complete.
            sync.dma_start(out=out_ext[:, :], in_=ACC[:, :]).then_inc(dma_sem, 16)

        @block.vector
        def _(v):
            Bk = B[:, :].rearrange("p (k c) -> p k c", c=4)     # [128,16,4]
            WHv = WH[:, :].rearrange("p (k c) -> p k c", c=2)   # [128,16,2]
            LTv = LT[:, :].rearrange("p (k c) -> p k c", c=2)   # [128,8,2]
            RBv = RB[:, :].rearrange("p (k c) -> p k c", c=2)

            v.wait_ge(dma_sem, 16)
            # lt = max(pred x1y1, target x1y1); rb = min(pred x2y2, target x2y2)
            v.tensor_tensor(LTv, Bk[:, 0:J, 0:2], Bk[:, J:16, 0:2], op=amax)
            v.tensor_tensor(RBv, Bk[:, 0:J, 2:4], Bk[:, J:16, 2:4], op=amin)
            # w,h for all 16 boxes (8 pred + 8 target) in one op
            v.tensor_tensor(WHv, Bk[:, :, 2:4], Bk[:, :, 0:2], op=sub)
            v.drain()
            v.tensor_tensor(D[:, :], RB[:, :], LT[:, :], op=sub)
            # areas for all 16 boxes
            v.tensor_tensor(AREA[:, :], WH[:, 0:32:2], WH[:, 1:32:2], op=mult)
            v.drain()
            v.tensor_relu(W[:, :], D[:, :])
            v.tensor_tensor(S[:, :], AREA[:, 0:J], AREA[:, J:16], op=add)
            v.drain()
            v.tensor_tensor(INTER[:, :], W[:, 0:16:2], W[:, 1:16:2], op=mult)
            v.drain()
            v.tensor_tensor(UNION[:, :], S[:, :], INTER[:, :], op=sub)
            v.drain()
            v.reciprocal(R[:, :], UNION[:, :])
            v.drain()
            v.tensor_tensor(IOU[:, :], INTER[:, :], R[:, :], op=mult)
            v.drain()
            # sq = iou*iou and per-partition acc[p] = sum_j sq in one fused op
            # (clip at eps dropped: changes the sum by <= 1e-14 per element,
            # below fp32 noise)
            v.scalar_tensor_tensor(
                SQ[:, :], IOU[:, :], 0.0, IOU[:, :],
                op0=byp, op1=mult, accum_out=ACC[:, :],
            )
            v.drain().then_inc(v_sem, 1)

    # CoreSim's race detector hardcodes 5 barrier participants, so sim
    # validation uses strip=False; the stripped graph is what runs on HW.
    return _strip_engines(nc) if strip else nc


_CACHE = {}


def _get_nc():
    if "nc" not in _CACHE:
        _CACHE["nc"] = build_bass()
    return _CACHE["nc"]


def make_in_maps(pred_boxes, target_boxes):
    p = np.ascontiguousarray(pred_boxes, dtype=np.float32).reshape(NCORES, P, 4 * J)
    t = np.ascontiguousarray(target_boxes, dtype=np.float32).reshape(NCORES, P, 4 * J)
    x = np.concatenate([p, t], axis=2)  # [8, 128, 64]
    return [{"x": np.ascontiguousarray(x[i])} for i in range(NCORES)]


def combine(results):
    total = np.float64(0.0)
    for r in results:
        total += np.float64(r["out"].sum(dtype=np.float64))
    return np.asarray(1.0 - total / N, dtype=np.float32) * np.float32(_SCALE)


def kernel(pred_boxes, target_boxes):
    nc = _get_nc()
    in_maps = make_in_maps(pred_boxes, target_boxes)
    res = run_bass_kernel_spmd(nc, in_maps, core_ids=list(range(NCORES)))
    return combine(res.results)
